# revision 14
# baseline (speedup 1.0000x reference)
"""Bahdanau (additive MLP) attention on 8 Trainium2 NeuronCores.

reference:
    q = query @ Wq.T            [B,M,H]
    k = memory @ Wm.T           [B,N,H]
    aligns[b,m,n] = w_out . tanh(q[b,m,:] + k[b,n,:])
    scores = softmax(aligns, axis=-1)
    out = scores @ memory       [B,M,D]

B, M, N, D, H = 4, 256, 512, 512, 512.

Sharding: core i handles batch b = i//2 and M-half i%2 (128 query rows).
Fully data-parallel -- softmax over N is local to a core. No collectives.

Per-core device algorithm (all tensors partition-major [128, F]):
  - q_projT[h, m] and k_projT[h, n] via PE matmuls (host pre-transposes
    Wq/Wm/memory/query so the contraction dim d is on partitions).
  - main loop over m (128) x h-chunk c (4):
      ACT: t[hp, n] = tanh(k_projT_c[hp, n] + bias q_projT_c[hp, m])
           (one activation instruction, free dim 512, bias fuses the
            broadcast add; tanh only exists on the scalar engine)
      PE:  alignsT[n_chunk j][:, m] += t[:, j*128:+128].T @ w_c  (over c)
  - softmax over n without max-subtraction (|aligns| <= ||w||_1 ~ 18,
    safe in f32): exp on ACT, sums + output matmul on PE, 1/s scale on DVE.
"""

import numpy as np

import concourse.tile as tile
from concourse import bacc, mybir
from concourse.bass_utils import run_bass_kernel_spmd

f32 = mybir.dt.float32
bf16 = mybir.dt.bfloat16
AF = mybir.ActivationFunctionType

B, M, N, D, H = 4, 256, 512, 512, 512
NCORES = 8
ML = M * B // NCORES  # 128 query rows per core

# "act_bias": fused bias-add inside the tanh activation (512 ACT instrs/core)
# "dve_add":  DVE pre-adds q+k, ACT does big-FD tanh (fewer ACT instrs)
MODE = "dve_add"
GRP = 4  # m-rows per ACT instruction in dve_add mode
KF32R = True  # float32r (single-pass, ~bf16-rounded inputs) for the k-projection


def _build(mode=MODE, grp=GRP, reps=1, kf32r=None, loop_loads=True):
    if kf32r is None:
        kf32r = KF32R
    f32r = mybir.dt.float32r
    nc = bacc.Bacc("TRN2", target_bir_lowering=False, debug=False, num_devices=NCORES)

    # DRAM inputs, already laid out partition-major by the host:
    # qT   [dp, (dc, m)]      = query[b, m0+m, dc*128+dp]
    # wqT  [dp, (dc, c, hp)]  = Wq[c*128+hp, dc*128+dp]
    # wmT  [dp, (dc, c, hp)]  = Wm[c*128+hp, dc*128+dp]
    # memT [dp, (dc, n)]      = memory[b, n, dc*128+dp]
    # memN [np_, (j, d)]      = memory[b, j*128+np_, d]
    # wo   [hp, c]            = w_out[c*128+hp]
    qT = nc.dram_tensor("qT", [128, 512], f32, kind="ExternalInput")
    wqT = nc.dram_tensor("wqT", [128, 2048], f32, kind="ExternalInput")
    kdt = f32r if kf32r else f32
    wmT = nc.dram_tensor("wmT", [128, 2048], kdt, kind="ExternalInput")
    memT = nc.dram_tensor("memT", [128, 2048], kdt, kind="ExternalInput")
    memN = nc.dram_tensor("memN", [128, 2048], f32, kind="ExternalInput")
    wo = nc.dram_tensor("wo", [128, 4], f32, kind="ExternalInput")
    eye = nc.dram_tensor("eye", [128, 128], f32, kind="ExternalInput")
    out = nc.dram_tensor("out", [128, 512], f32, kind="ExternalOutput")

    with tile.TileContext(nc) as tc:
        with (
            tc.tile_pool(name="const", bufs=1) as const,
            tc.tile_pool(name="tbuf", bufs=8 if mode == "act_bias" else 2) as tpool,
            tc.tile_pool(name="zbuf", bufs=2) as zpool,
            tc.tile_pool(name="kps", bufs=4, space="PSUM") as kpool,
            tc.tile_pool(name="alps", bufs=1, space="PSUM") as apool,
            tc.tile_pool(name="misc", bufs=2, space="PSUM") as mpool,
        ):
            wo_sb = const.tile([128, 4], f32)
            qT_sb = const.tile([128, 512], f32)
            wqT_sb = const.tile([128, 2048], f32)
            wmT_sb = const.tile([128, 2048], kdt)
            memT_sb = const.tile([128, 2048], kdt)
            memN_sb = const.tile([128, 2048], f32)
            eye_sb = const.tile([128, 128], f32)
            warm_sb = const.tile([128, 128], f32)

            def loads():
                # Preload the ACT spline table set while DMAs stream: the
                # first Tanh otherwise pays the ~2.7us ACT_TABLE_LOAD on the
                # critical path (exp_and_others covers both Tanh and Exp).
                nc.vector.memset(warm_sb[:], 1.0)
                nc.scalar.activation(warm_sb[:, 0:1], warm_sb[:, 0:1], AF.Tanh)
                # DMA order: k inputs as (wmT, memT) per-dc pairs first so
                # the k-projection pipelines, then the q path, memN
                # (epilogue-only) last.
                nc.sync.dma_start(wo_sb[:], wo.ap())
                if mode == "dve_add":
                    nc.vector.tensor_copy(wob_sb[:], wo_sb[:])
                nc.sync.dma_start(eye_sb[:], eye.ap())
                nc.vector.memset(ones_col[:], 1.0)
                for dc in range(4):
                    sl = slice(dc * 512, (dc + 1) * 512)
                    nc.sync.dma_start(wmT_sb[:, sl], wmT.ap()[:, sl])
                    nc.sync.dma_start(memT_sb[:, sl], memT.ap()[:, sl])
                nc.sync.dma_start(qT_sb[:], qT.ap())
                nc.sync.dma_start(wqT_sb[:], wqT.ap())
                nc.sync.dma_start(memN_sb[:], memN.ap())
                # PE warm-up: the HAM clock gate holds PE at 1.2 GHz until it
                # has been busy ~3.4us. Burn that window on dummy matmuls
                # while the input DMAs stream, so the projection matmuls run
                # at 2.4 GHz.
                warm_ps = mpool.tile([128, 128], f32, tag="misc")
                for _ in range(14):
                    nc.tensor.matmul(
                        warm_ps[:], warm_sb[:], warm_sb[:], start=True, stop=True,
                    )

            q_sb = const.tile([128, 512], f32)  # [hp, (c, m)]
            if mode == "dve_add":
                k_sb = const.tile([128, 2048], f32)  # [hp, (c, n)]
                # bf16 stationary + bf16 moving for the h-reduction matmuls:
                # fp32 stationary loads are ~4x slower (no fast weight load),
                # which would make PE the bottleneck. tanh in [-1,1] loses
                # only ~2e-3 absolute in bf16.
                wob_sb = const.tile([128, 4], bf16)
            exp_sb = const.tile([128, 512], f32)  # [n_sub, (j, m)]
            ones_col = const.tile([128, 1], f32)
            rs_sb = const.tile([128, 1], f32)
            out_sb = const.tile([128, 512], f32)

            def body():
                # k_projT[h, n]: per h-chunk c, accumulate over d-chunks.
                # Resident in PSUM (act_bias reads it there); for dve_add
                # copied to SBUF via the (idle at this point) scalar engine.
                # dc-outer: each bank c holds its own open accumulation
                # group, so the dc<3 matmuls pipeline with the DMA pairs
                # instead of waiting for the last one.
                k_ps = [kpool.tile([128, 512], f32, tag="k", name=f"kp{i}") for i in range(4)]
                for dc in range(4):
                    for c in range(4):
                        nc.tensor.matmul(
                            k_ps[c][:],
                            wmT_sb[:, dc * 512 + c * 128 : dc * 512 + (c + 1) * 128],
                            memT_sb[:, dc * 512 : (dc + 1) * 512],
                            start=(dc == 0),
                            stop=(dc == 3),
                        )
                if mode == "dve_add":
                    for c in range(4):
                        nc.scalar.copy(k_sb[:, c * 512 : (c + 1) * 512], k_ps[c][:])

                # q_projT[h, m] (inputs arrive after the k inputs; PE does the
                # k groups first, these run while wqT finishes streaming)
                for c in range(4):
                    qp = mpool.tile([128, 128], f32, tag="misc")
                    for dc in range(4):
                        nc.tensor.matmul(
                            qp[:],
                            wqT_sb[:, dc * 512 + c * 128 : dc * 512 + (c + 1) * 128],
                            qT_sb[:, dc * 128 : (dc + 1) * 128],
                            start=(dc == 0),
                            stop=(dc == 3),
                        )
                    nc.vector.tensor_copy(q_sb[:, c * 128 : (c + 1) * 128], qp[:])
                return k_ps

            def main_and_epilogue(k_ps):

                # alignsT [n_sub, (j, m)] accumulated in one PSUM bank.
                # Groups must be sequential within a bank (start=True clears
                # has_written for the whole bank), hence j-outer / c-inner.
                al = apool.tile([128, 512], f32)

                if mode == "act_bias":
                    for m in range(ML):
                        ts = []
                        for c in range(4):
                            t = tpool.tile([128, 512], f32, tag="t")
                            nc.scalar.activation(
                                t[:],
                                k_ps[c][:],
                                AF.Tanh,
                                bias=q_sb[:, c * 128 + m : c * 128 + m + 1],
                                scale=1.0,
                            )
                            ts.append(t)
                        for j in range(4):
                            for c in range(4):
                                nc.tensor.matmul(
                                    al[:, j * 128 + m : j * 128 + m + 1],
                                    ts[c][:, j * 128 : (j + 1) * 128],
                                    wo_sb[:, c : c + 1],
                                    start=(c == 0),
                                    stop=(c == 3),
                                )
                else:  # dve_add
                    # First few rows act_bias-style straight from PSUM k
                    # (no z dependency) while DVE builds the first z tiles.
                    ramp = min(2, ML) if grp > 1 else 0
                    for m in range(ramp):
                        ts = []
                        for c in range(4):
                            t = tpool.tile([128, 512], bf16, tag="tr", bufs=8)
                            nc.scalar.activation(
                                t[:, 0:512],
                                k_ps[c][:],
                                AF.Tanh,
                                bias=q_sb[:, c * 128 + m : c * 128 + m + 1],
                                scale=1.0,
                            )
                            ts.append(t)
                        for j in range(4):
                            for c in range(4):
                                nc.tensor.matmul(
                                    al[:, j * 128 + m : j * 128 + m + 1],
                                    ts[c][:, j * 128 : (j + 1) * 128],
                                    wob_sb[:, c : c + 1],
                                    start=(c == 0),
                                    stop=(c == 3),
                                )
                    plan = []
                    m0 = ramp
                    while m0 < ML:
                        g = min(grp, ML - m0)
                        plan.append((m0, g))
                        m0 += g
                    for m0, g in plan:
                        z = zpool.tile([128, 2048 * grp], f32, tag="z")
                        for c in range(4):
                            for gi in range(g):
                                m = m0 + gi
                                nc.vector.tensor_scalar_add(
                                    z[:, gi * 2048 + c * 512 : gi * 2048 + (c + 1) * 512],
                                    k_sb[:, c * 512 : (c + 1) * 512],
                                    q_sb[:, c * 128 + m : c * 128 + m + 1],
                                )
                        t = tpool.tile([128, 2048 * grp], bf16, tag="t")
                        nc.scalar.activation(t[:, : 2048 * g], z[:, : 2048 * g], AF.Tanh)
                        for gi in range(g):
                            m = m0 + gi
                            for j in range(4):
                                for c in range(4):
                                    nc.tensor.matmul(
                                        al[:, j * 128 + m : j * 128 + m + 1],
                                        t[
                                            :,
                                            gi * 2048 + c * 512 + j * 128 : gi * 2048
                                            + c * 512
                                            + (j + 1) * 128,
                                        ],
                                        wob_sb[:, c : c + 1],
                                        start=(c == 0),
                                        stop=(c == 3),
                                    )

                # softmax over n (no max subtraction; |aligns| <= ~18).
                # aligns is [n_sub, (j, m)]; exp on ACT, n-sums + output
                # matmul on PE, 1/s scale on DVE.
                nc.scalar.activation(exp_sb[:], al[:], AF.Exp)

                s_ps = mpool.tile([128, 1], f32, tag="misc")
                for j in range(4):
                    nc.tensor.matmul(
                        s_ps[:],
                        exp_sb[:, j * 128 : (j + 1) * 128],
                        ones_col[:, 0:1],
                        start=(j == 0),
                        stop=(j == 3),
                    )
                nc.vector.reciprocal(rs_sb[:], s_ps[:])

                o_ps = mpool.tile([128, 512], f32, tag="misc")
                for j in range(4):
                    nc.tensor.matmul(
                        o_ps[:],
                        exp_sb[:, j * 128 : (j + 1) * 128],
                        memN_sb[:, j * 512 : (j + 1) * 512],
                        start=(j == 0),
                        stop=(j == 3),
                    )
                nc.vector.tensor_scalar_mul(out_sb[:], o_ps[:], rs_sb[:])
                nc.sync.dma_start(out.ap(), out_sb[:])

            if reps == 1:
                loads()
                main_and_epilogue(body())
            elif loop_loads:
                with tc.For_i(0, reps, 1):
                    loads()
                    main_and_epilogue(body())
            else:
                loads()
                with tc.For_i(0, reps, 1):
                    main_and_epilogue(body())

    nc.compile()
    return nc


_nc_cache = {}


def _get_nc(mode=MODE, grp=GRP):
    key = (mode, grp, KF32R)
    if key not in _nc_cache:
        _nc_cache[key] = _build(mode, grp)
    return _nc_cache[key]


def _shard_inputs(query, memory, Wq, Wm, w_out):
    query = np.ascontiguousarray(query, dtype=np.float32)
    memory = np.ascontiguousarray(memory, dtype=np.float32)
    Wq = np.ascontiguousarray(Wq, dtype=np.float32)
    Wm = np.ascontiguousarray(Wm, dtype=np.float32)
    w_out = np.ascontiguousarray(w_out, dtype=np.float32)

    # [dp, (dc, c, hp)]
    wqT_h = np.ascontiguousarray(
        Wq.T.reshape(4, 128, 4, 128).transpose(1, 0, 2, 3).reshape(128, 2048)
    )
    wmT_h = np.ascontiguousarray(
        Wm.T.reshape(4, 128, 4, 128).transpose(1, 0, 2, 3).reshape(128, 2048)
    )
    wo_h = np.ascontiguousarray(w_out.reshape(4, 128).T)  # [hp, c]
    eye_h = np.eye(128, dtype=np.float32)

    in_maps = []
    for i in range(NCORES):
        b, mh = divmod(i, 2)
        qT_h = np.ascontiguousarray(
            query[b, mh * ML : (mh + 1) * ML, :]
            .T.reshape(4, 128, 128)
            .transpose(1, 0, 2)
            .reshape(128, 512)
        )
        memT_h = np.ascontiguousarray(
            memory[b].T.reshape(4, 128, 512).transpose(1, 0, 2).reshape(128, 2048)
        )
        memN_h = np.ascontiguousarray(
            memory[b].reshape(4, 128, 512).transpose(1, 0, 2).reshape(128, 2048)
        )
        in_maps.append(
            {
                "qT": qT_h,
                "wqT": wqT_h,
                "wmT": wmT_h,
                "memT": memT_h,
                "memN": memN_h,
                "wo": wo_h,
                "eye": eye_h,
            }
        )
    return in_maps


def kernel(query, memory, Wq, Wm, w_out):
    nc = _get_nc()
    in_maps = _shard_inputs(query, memory, Wq, Wm, w_out)
    res = run_bass_kernel_spmd(nc, in_maps, core_ids=list(range(NCORES)))
    full = np.empty((B, M, D), dtype=np.float32)
    for i in range(NCORES):
        b, mh = divmod(i, 2)
        full[b, mh * ML : (mh + 1) * ML, :] = res.results[i]["out"]
    return full

